# revision 12
# baseline (speedup 1.0000x reference)
"""Grouped-KV attention block (dense transformer) on 8 Trainium2 NeuronCores.

Sharding (Megatron-style, per the hint): data-parallel over batch (2) x
tensor-parallel over KV-head groups (4).  core = b*4 + g owns batch b and
KV heads {2g, 2g+1} with their 8 query heads (Wq/Wkv column-sharded,
Wo row-sharded).  Each core produces a partial [L, HID] output; the host
sums the 4 partials per batch and adds bo (the row-parallel reduction).

Per-core kernel (all matmuls on TensorE, fp32 data, float32r compute;
the probs@V matmul runs in bf16 because its free dim is 129):
  A) PE-transpose X -> X^T tiles; project:  Q^T = Wq^T X^T (spilled to
     DRAM), K^T = Wk^T X^T, V = X Wv (+bias via ones-row matmul).
  B) per (head, 512-query block): S^T = K Q^T per 128-key chunk (PSUM),
     exp on ScalarE -> bf16 probs, O[q,129] += P^T.T @ [V | 1] --
     column 128 accumulates the softmax denominator.  Normalize rows by
     1/denom, PE-transpose to O^T.
  C) Y = O @ Wo (accumulate over the 8 local head chunks), DMA out.
"""

import math

import numpy as np

P = 128
HID = 4096
L = 2048
NH_LOC = 8  # query heads per core
NKV_LOC = 2  # kv heads per core
HD = 128  # head dim
G_LOC = NH_LOC // NKV_LOC
SB = 512  # L superblock / matmul free dim
HC = HID // P  # hid chunks (32)
LC = L // P  # L chunks (16)
NSB = L // SB
SBC = SB // P
N_CORES = 8


# ---------------------------------------------------------------------------
# TileContext with the tail-drain waits split across multiple drains.
# This walrus build rejects >2 sync waits on a CTRL(Drain) instruction
# ("Too many sync wait commands"); Tile attaches the full global-clock
# wait set to the single tail drain.  Split: one drain per wait.
# ---------------------------------------------------------------------------
def _make_tc_class():
    import concourse.tile as tile
    from concourse.vector_clock import ScopedClock

    class SplitDrainTileContext(tile.TileContext):
        def _drain_and_barrier(self, tick_clock, wait_clock):
            nc = self.nc
            drain_inst = nc.sync.drain()
            wait_clock.add_sem_waits(
                drain_inst.ins, ScopedClock({None: tick_clock.global_clock})
            )
            si = drain_inst.ins.sync_info
            waits = list(si.on_wait) if si and si.on_wait else []
            if len(waits) > 1:
                si.on_wait = waits[:1]
                for w in waits[1:]:
                    extra = nc.sync.drain()
                    esi = extra.ins.sync_info
                    if esi is None:
                        import concourse.mybir as mybir

                        extra.ins.sync_info = mybir.SyncInfo(
                            on_wait=[w], on_update=[]
                        )
                    else:
                        esi.on_wait = [w]

            nc.all_engine_barrier()
            assert self.sems is not None
            popped = nc._tile_sem_poison_stack.pop()
            assert popped is self._sem_poison
            nc.clear_and_free_semaphores(list(self.sems.allocated().values()))
            nc.all_engine_barrier()

    return SplitDrainTileContext


def build_attention_kernel(
    L=L,
    HID=HID,
    NH_LOC=NH_LOC,
    NKV_LOC=NKV_LOC,
    HD=HD,
    mm_dtype="float32r",
    av_dtype="bfloat16",
):
    """Build the per-core Bass module.  Returns (nc, names) where names
    maps logical tensors to dram tensor names."""
    import concourse.bass as bass
    import concourse.mybir as mybir
    from concourse.masks import make_identity
    from contextlib import ExitStack

    assert L % SB == 0 and HID % P == 0 and HD == P
    HC = HID // P
    LC = L // P
    NSB = L // SB
    SBC = SB // P
    KC = L // P  # key chunks
    G = NH_LOC // NKV_LOC
    f32 = mybir.dt.float32
    mmdt = getattr(mybir.dt, mm_dtype)
    avdt = getattr(mybir.dt, av_dtype)
    # bf16 probs run the AV matmul at its natural width (129).  fp32/f32r
    # probs pad the moving operand to 256 so float32r streams at full rate.
    AVW = HD + 1 if av_dtype == "bfloat16" else 2 * HD
    scale = 1.0 / math.sqrt(HD)

    import concourse.bacc as bacc
    import concourse.tile as tile

    TC = tile.TileContext
    nc = bacc.Bacc("TRN2", target_bir_lowering=False, debug=False,
                   num_devices=N_CORES)

    xq_d = nc.dram_tensor("xq", [L, HID], f32, kind="ExternalInput")
    xkv_d = nc.dram_tensor("xkv", [L, HID], f32, kind="ExternalInput")
    # weights pre-rearranged on host: wq [NH, P, HC, HD]; wk [NKV, P, HC, HD];
    # wv [P, HC, NKV*HD]; wo [HID//SB, P, NH, SB]
    wq_d = nc.dram_tensor("wq", [NH_LOC, P, HC, HD], mmdt, kind="ExternalInput")
    wk_d = nc.dram_tensor("wk", [NKV_LOC, P, HC, HD], mmdt, kind="ExternalInput")
    wv_d = nc.dram_tensor("wv", [P, HC, NKV_LOC * HD], mmdt, kind="ExternalInput")
    wo_d = nc.dram_tensor("wo", [HID // SB, P, NH_LOC, SB], mmdt,
                          kind="ExternalInput")
    bqc_d = nc.dram_tensor("bqc", [P, NH_LOC], f32, kind="ExternalInput")
    bkc_d = nc.dram_tensor("bkc", [P, NKV_LOC], f32, kind="ExternalInput")
    bvp_d = nc.dram_tensor("bvp", [P, NKV_LOC * HD], mmdt, kind="ExternalInput")
    ones_d = nc.dram_tensor("ones", [P, P], mmdt, kind="ExternalInput")
    y_d = nc.dram_tensor("y", [L, HID], f32, kind="ExternalOutput")
    qt_spill = nc.dram_tensor("qt_spill", [NH_LOC, HD, L], mmdt)

    def mm(out, lhsT, rhs, start, stop, dt=mmdt):
        nc.tensor.matmul(
            out,
            lhsT.bitcast(dt) if dt != lhsT.dtype else lhsT,
            rhs.bitcast(dt) if dt != rhs.dtype else rhs,
            start=start,
            stop=stop,
        )

    with TC(nc) as tc, ExitStack() as top:
        consts = top.enter_context(tc.tile_pool(name="consts", bufs=1))
        persist = top.enter_context(tc.tile_pool(name="persist", bufs=1))

        ident = consts.tile([P, P], f32)
        make_identity(nc, ident)
        ones_t = consts.tile([P, P], mmdt)
        nc.sync.dma_start(ones_t[:], ones_d[:])
        bqc = consts.tile([P, NH_LOC], f32)
        nc.sync.dma_start(bqc[:], bqc_d[:])
        bkc = consts.tile([P, NKV_LOC], f32)
        nc.sync.dma_start(bkc[:], bkc_d[:])
        bvp = consts.tile([P, NKV_LOC * HD], mmdt)
        nc.sync.dma_start(bvp[:], bvp_d[:])

        kt_sb = persist.tile([P, NKV_LOC, L], mmdt, tag="kt")
        vaug = persist.tile([P, NKV_LOC, KC, AVW], avdt, tag="vaug")
        if AVW > HD + 1:
            nc.gpsimd.memset(vaug[:, :, :, HD + 1 :], 0.0)
        nc.gpsimd.memset(vaug[:, :, :, HD : HD + 1], 1.0)

        # ------------------------------------------------------------------
        # Phase A: transposes + Q/K/V projections (per L-superblock)
        # ------------------------------------------------------------------
        with ExitStack() as pa:
            xt_p = pa.enter_context(tc.tile_pool(name="xt", bufs=1))
            xnat_p = pa.enter_context(tc.tile_pool(name="xnat", bufs=3))
            w_p = pa.enter_context(tc.tile_pool(name="wstream", bufs=2))
            bounce_p = pa.enter_context(tc.tile_pool(name="bounce", bufs=2))
            psA = pa.enter_context(
                tc.tile_pool(name="psA", bufs=1, space="PSUM")
            )

            def transpose_superblock(src_dram, s, xt):
                # xt: [P, HC, SB] <- src[s*SB:(s+1)*SB, :].T
                for lc2 in range(SBC):
                    row0 = (s * SBC + lc2) * P
                    for hq in range(HC // 4):
                        xn = xnat_p.tile([P, 4 * P], f32, tag="xn")
                        nc.sync.dma_start(
                            xn[:],
                            src_dram[row0 : row0 + P,
                                     hq * 4 * P : (hq + 1) * 4 * P],
                        )
                        tp = psA.tile([P, 4 * P], f32, tag="tp", bufs=2)
                        for hh in range(4):
                            nc.tensor.matmul(
                                tp[:, hh * P : (hh + 1) * P],
                                xn[:, hh * P : (hh + 1) * P],
                                ident,
                                is_transpose=True,
                                start=True,
                                stop=True,
                            )
                        nc.vector.tensor_copy(
                            xt[:, hq * 4 : (hq + 1) * 4,
                               lc2 * P : (lc2 + 1) * P],
                            tp[:].rearrange("p (c f) -> p c f", c=4),
                        )

            for s in range(NSB):
                xt = xt_p.tile([P, HC, SB], mmdt, tag="xt")
                transpose_superblock(xq_d, s, xt)
                # Q projection -> Q^T spilled to DRAM
                for h in range(NH_LOC):
                    wqt = w_p.tile([P, HC, HD], mmdt, tag="w")
                    nc.sync.dma_start(wqt[:], wq_d[h])
                    qtp = psA.tile([P, SB], f32, tag="acc", bufs=2)
                    for hc in range(HC):
                        mm(qtp, wqt[:, hc, :], xt[:, hc, :],
                           start=(hc == 0), stop=(hc == HC - 1))
                    qts = bounce_p.tile([P, SB], mmdt, tag="qts")
                    nc.scalar.activation(
                        qts, qtp,
                        mybir.ActivationFunctionType.Identity,
                        bias=bqc[:, h : h + 1],
                    )
                    nc.sync.dma_start(
                        qt_spill[h, :, s * SB : (s + 1) * SB], qts
                    )

                xt2 = xt_p.tile([P, HC, SB], mmdt, tag="xt")
                transpose_superblock(xkv_d, s, xt2)
                # K projection -> K^T resident
                for j in range(NKV_LOC):
                    wkt = w_p.tile([P, HC, HD], mmdt, tag="w")
                    nc.sync.dma_start(wkt[:], wk_d[j])
                    ktp = psA.tile([P, SB], f32, tag="acc", bufs=2)
                    for hc in range(HC):
                        mm(ktp, wkt[:, hc, :], xt2[:, hc, :],
                           start=(hc == 0), stop=(hc == HC - 1))
                    nc.scalar.activation(
                        kt_sb[:, j, s * SB : (s + 1) * SB], ktp,
                        mybir.ActivationFunctionType.Identity,
                        bias=bkc[:, j : j + 1],
                    )
                # V projection (natural layout) + bias via ones-row matmul
                wvt = w_p.tile([P, HC, NKV_LOC * HD], mmdt, tag="w")
                nc.sync.dma_start(wvt[:], wv_d[:])
                for lc2 in range(SBC):
                    vp = psA.tile([P, NKV_LOC * HD], f32, tag="acc", bufs=2)
                    mm(vp, ones_t[:], bvp[:], start=True, stop=False)
                    for hc in range(HC):
                        mm(vp, xt2[:, hc, lc2 * P : (lc2 + 1) * P],
                           wvt[:, hc, :],
                           start=False, stop=(hc == HC - 1))
                    kc = s * SBC + lc2
                    for j in range(NKV_LOC):
                        nc.vector.tensor_copy(
                            vaug[:, j, kc, 0:HD],
                            vp[:, j * HD : (j + 1) * HD],
                        )

        # ------------------------------------------------------------------
        # Phase B: attention per (head, 512-query block)
        # ------------------------------------------------------------------
        ot_pool = top.enter_context(tc.tile_pool(name="otp", bufs=1))
        ot_sb = ot_pool.tile([P, NH_LOC, L], mmdt, tag="ot")
        with ExitStack() as pb:
            qt_p = pb.enter_context(tc.tile_pool(name="qth", bufs=2))
            pt_p = pb.enter_context(tc.tile_pool(name="pt", bufs=3))
            osb_p = pb.enter_context(tc.tile_pool(name="osb", bufs=3))
            rcp_p = pb.enter_context(tc.tile_pool(name="rcp", bufs=3))
            psB = pb.enter_context(
                tc.tile_pool(name="psB", bufs=1, space="PSUM")
            )

            QBLK = 512
            NQB = L // QBLK
            QS = QBLK // P
            for h in range(NH_LOC):
                j = h // G
                qth = qt_p.tile([P, L], mmdt, tag="qth")
                nc.sync.dma_start(qth[:], qt_spill[h])
                for qb in range(NQB):
                    o_ps = [
                        psB.tile([P, AVW], f32, tag=f"o{qs}",
                                 name=f"o_ps{qs}")
                        for qs in range(QS)
                    ]
                    for kc in range(KC):
                        stp = psB.tile([P, QBLK], f32, tag="st", bufs=2)
                        mm(stp, kt_sb[:, j, kc * P : (kc + 1) * P],
                           qth[:, qb * QBLK : (qb + 1) * QBLK],
                           start=True, stop=True)
                        pt = pt_p.tile([P, QBLK], avdt, tag="pt")
                        nc.scalar.activation(
                            pt, stp,
                            mybir.ActivationFunctionType.Exp,
                            scale=scale,
                        )
                        for qs in range(QS):
                            mm(o_ps[qs], pt[:, qs * P : (qs + 1) * P],
                               vaug[:, j, kc, :],
                               start=(kc == 0), stop=(kc == KC - 1),
                               dt=avdt)
                    tps = psB.tile([P, QBLK], f32, tag="ott")
                    for qs in range(QS):
                        rcp = rcp_p.tile([P, 1], f32, tag="rcp")
                        nc.vector.reciprocal(
                            rcp, o_ps[qs][:, HD : HD + 1]
                        )
                        osb = osb_p.tile([P, HD], f32, tag="osb")
                        nc.vector.tensor_scalar_mul(
                            osb, o_ps[qs][:, 0:HD], rcp
                        )
                        nc.tensor.matmul(
                            tps[:, qs * P : (qs + 1) * P],
                            osb, ident,
                            is_transpose=True, start=True, stop=True,
                        )
                    nc.vector.tensor_copy(
                        ot_sb[:, h, qb * QBLK : (qb + 1) * QBLK], tps
                    )

        # ------------------------------------------------------------------
        # Phase C: Y = O @ Wo  (partial output; host sums across cores)
        # ------------------------------------------------------------------
        with ExitStack() as pc:
            wo_p = pc.enter_context(tc.tile_pool(name="wo", bufs=2))
            y_p = pc.enter_context(tc.tile_pool(name="ysb", bufs=3))
            psC = pc.enter_context(
                tc.tile_pool(name="psC", bufs=2, space="PSUM")
            )
            for hb in range(HID // SB):
                wot = wo_p.tile([P, NH_LOC, SB], mmdt, tag="wo")
                nc.sync.dma_start(wot[:], wo_d[hb])
                for qc in range(LC):
                    yp = psC.tile([P, SB], f32, tag="y")
                    for h in range(NH_LOC):
                        mm(yp, ot_sb[:, h, qc * P : (qc + 1) * P],
                           wot[:, h, :],
                           start=(h == 0), stop=(h == NH_LOC - 1))
                    ysb = y_p.tile([P, SB], f32, tag="y")
                    nc.any.tensor_copy(ysb, yp)
                    nc.sync.dma_start(
                        y_d[qc * P : (qc + 1) * P,
                            hb * SB : (hb + 1) * SB],
                        ysb,
                    )

    nc.compile()
    return nc


# ---------------------------------------------------------------------------
# Host-side sharding / unsharding
# ---------------------------------------------------------------------------
def shard_inputs(query, kv, Wq, bq, Wkv, bkv, Wo, bo):
    """Slice + rearrange the full inputs into 8 per-core input maps."""
    NDQ = NH_LOC * HD  # 1024 local q dims
    NDK = NKV_LOC * HD  # 256 local kv dims
    in_maps = []
    for core in range(N_CORES):
        b, g = core // 4, core % 4
        wq_s = np.ascontiguousarray(Wq[:, g * NDQ : (g + 1) * NDQ])
        wk_s = np.ascontiguousarray(Wkv[:, g * NDK : (g + 1) * NDK])
        wv_s = np.ascontiguousarray(
            Wkv[:, NKV_LOC * 4 * HD + g * NDK : NKV_LOC * 4 * HD + (g + 1) * NDK]
        )
        wo_s = np.ascontiguousarray(Wo[g * NDQ : (g + 1) * NDQ, :])
        bq_s = bq[g * NDQ : (g + 1) * NDQ]
        bk_s = bkv[g * NDK : (g + 1) * NDK]
        bv_s = bkv[NKV_LOC * 4 * HD + g * NDK : NKV_LOC * 4 * HD + (g + 1) * NDK]

        wq_r = np.ascontiguousarray(
            wq_s.reshape(HC, P, NH_LOC, HD).transpose(2, 1, 0, 3)
        )
        wk_r = np.ascontiguousarray(
            wk_s.reshape(HC, P, NKV_LOC, HD).transpose(2, 1, 0, 3)
        )
        wv_r = np.ascontiguousarray(wv_s.reshape(HC, P, NDK).transpose(1, 0, 2))
        wo_r = np.ascontiguousarray(
            wo_s.reshape(NH_LOC, P, HID // SB, SB).transpose(2, 1, 0, 3)
        )
        bqc = np.ascontiguousarray(bq_s.reshape(NH_LOC, P).T)
        bkc = np.ascontiguousarray(bk_s.reshape(NKV_LOC, P).T)
        bvp = np.zeros((P, NDK), np.float32)
        bvp[0, :] = bv_s

        in_maps.append(
            {
                "xq": np.ascontiguousarray(query[b]),
                "xkv": np.ascontiguousarray(kv[b]),
                "wq": wq_r,
                "wk": wk_r,
                "wv": wv_r,
                "wo": wo_r,
                "bqc": bqc,
                "bkc": bkc,
                "bvp": bvp,
                "ones": np.ones((P, P), np.float32),
            }
        )
    return in_maps


def unshard_output(results, bo):
    """Sum the 4 row-parallel partials per batch and add bo."""
    out = np.empty((2, L, HID), np.float32)
    for b in range(2):
        acc = results[b * 4]["y"].astype(np.float32)
        for g in range(1, 4):
            acc = acc + results[b * 4 + g]["y"]
        out[b] = acc + bo.astype(np.float32)[None, :]
    return out


_NC_CACHE = {}


def run_sharded(query, kv, Wq, bq, Wkv, bkv, Wo, bo, trace=False, tmpdir=None,
                **build_kwargs):
    """Shard, run on 8 cores, unshard.  Returns (output, BassKernelResults)."""
    from concourse.bass_utils import run_bass_kernel_spmd

    key = tuple(sorted(build_kwargs.items()))
    if key not in _NC_CACHE:
        _NC_CACHE[key] = build_attention_kernel(**build_kwargs)
    nc = _NC_CACHE[key]

    in_maps = shard_inputs(
        np.asarray(query, np.float32),
        np.asarray(kv, np.float32),
        np.asarray(Wq, np.float32),
        np.asarray(bq, np.float32),
        np.asarray(Wkv, np.float32),
        np.asarray(bkv, np.float32),
        np.asarray(Wo, np.float32),
        np.asarray(bo, np.float32),
    )
    res = run_bass_kernel_spmd(
        nc, in_maps, list(range(N_CORES)), trace=trace, tmpdir=tmpdir
    )
    return unshard_output(res.results, np.asarray(bo, np.float32)), res


def kernel(query, kv, Wq, bq, Wkv, bkv, Wo, bo):
    out, _ = run_sharded(query, kv, Wq, bq, Wkv, bkv, Wo, bo)
    return out


# revision 14
# speedup vs baseline: 1.3626x; 1.3626x over previous
"""Grouped-KV attention block (dense transformer) on 8 Trainium2 NeuronCores.

Sharding (Megatron-style, per the hint): data-parallel over batch (2) x
tensor-parallel over KV-head groups (4).  core = b*4 + g owns batch b and
KV heads {2g, 2g+1} with their 8 query heads (Wq/Wkv column-sharded,
Wo row-sharded).  Each core produces a partial [L, HID] output; the host
sums the 4 partials per batch and adds bo (the row-parallel reduction).

Per-core kernel, bf16 operands / fp32 PSUM accumulation throughout
(4-byte matmul operands stream at half rate on TRN2, so fp32/f32r
compute leaves ~2x on the table):
  A) X^T via XBAR DMA-transpose of host-precast bf16 X (no PE/DVE work);
     Q^T = Wq^T X^T (resident), K^T = Wk^T X^T, V = X Wv (+bias via a
     ones-row matmul).
  B) per (head, 512-query block): S^T = K Q^T per 128-key chunk (PSUM
     fp32), exp on ScalarE -> bf16 probs, O[q,129] += P^T.T @ [V | 1] --
     column 128 accumulates the softmax denominator.  Normalize rows by
     1/denom (per-partition scalar), PE-transpose to O^T.
  C) Y = O @ Wo (accumulate over the 8 local head chunks), DMA out fp32.
"""

import math

import numpy as np

P = 128
HID = 4096
L = 2048
NH_LOC = 8  # query heads per core
NKV_LOC = 2  # kv heads per core
HD = 128  # head dim
SB = 512  # L superblock / matmul free dim
N_CORES = 8


def build_attention_kernel(L=L, HID=HID, NH_LOC=NH_LOC, NKV_LOC=NKV_LOC,
                           HD=HD):
    """Build the per-core Bacc module (compiled, ready for SPMD run)."""
    import concourse.bacc as bacc
    import concourse.mybir as mybir
    import concourse.tile as tile
    from contextlib import ExitStack

    assert L % SB == 0 and HID % P == 0 and HD == P
    HC = HID // P
    LC = L // P
    NSB = L // SB
    SBC = SB // P
    KC = L // P  # key chunks
    G = NH_LOC // NKV_LOC
    f32 = mybir.dt.float32
    bf16 = mybir.dt.bfloat16
    AVW = HD + 1
    scale = 1.0 / math.sqrt(HD)

    nc = bacc.Bacc("TRN2", target_bir_lowering=False, debug=False,
                   num_devices=N_CORES)

    # host-precast bf16 activations and pre-rearranged bf16 weights:
    #   wq [NH, P, HC, HD]; wk [NKV, P, HC, HD]; wv [P, HC, NKV*HD];
    #   wo [HID//SB, P, NH, SB]
    xqb_d = nc.dram_tensor("xqb", [L, HID], bf16, kind="ExternalInput")
    xkvb_d = nc.dram_tensor("xkvb", [L, HID], bf16, kind="ExternalInput")
    wq_d = nc.dram_tensor("wq", [NH_LOC, P, HC, HD], bf16, kind="ExternalInput")
    wk_d = nc.dram_tensor("wk", [NKV_LOC, P, HC, HD], bf16,
                          kind="ExternalInput")
    wv_d = nc.dram_tensor("wv", [P, HC, NKV_LOC * HD], bf16,
                          kind="ExternalInput")
    wo_d = nc.dram_tensor("wo", [HID // SB, P, NH_LOC, SB], bf16,
                          kind="ExternalInput")
    bqc_d = nc.dram_tensor("bqc", [P, NH_LOC], f32, kind="ExternalInput")
    bkc_d = nc.dram_tensor("bkc", [P, NKV_LOC], f32, kind="ExternalInput")
    bvp_d = nc.dram_tensor("bvp", [P, NKV_LOC * HD], bf16,
                           kind="ExternalInput")
    ones_d = nc.dram_tensor("ones", [P, P], bf16, kind="ExternalInput")
    ident_d = nc.dram_tensor("ident", [P, P], bf16, kind="ExternalInput")
    y_d = nc.dram_tensor("y", [L, HID], f32, kind="ExternalOutput")

    def mm(out, lhsT, rhs, start, stop):
        nc.tensor.matmul(out, lhsT, rhs, start=start, stop=stop)

    with tile.TileContext(nc) as tc, ExitStack() as top:
        consts = top.enter_context(tc.tile_pool(name="consts", bufs=1))
        persist = top.enter_context(tc.tile_pool(name="persist", bufs=1))

        ident = consts.tile([P, P], bf16)
        nc.sync.dma_start(ident[:], ident_d[:])
        ones_t = consts.tile([P, P], bf16)
        nc.sync.dma_start(ones_t[:], ones_d[:])
        bqc = consts.tile([P, NH_LOC], f32)
        nc.sync.dma_start(bqc[:], bqc_d[:])
        bkc = consts.tile([P, NKV_LOC], f32)
        nc.sync.dma_start(bkc[:], bkc_d[:])
        bvp = consts.tile([P, NKV_LOC * HD], bf16)
        nc.sync.dma_start(bvp[:], bvp_d[:])

        kt_sb = persist.tile([P, NKV_LOC, L], bf16, tag="kt")
        vaug = persist.tile([P, NKV_LOC, KC, AVW], bf16, tag="vaug")
        nc.gpsimd.memset(vaug[:, :, :, HD : HD + 1], 1.0)
        qt_sb = persist.tile([P, NH_LOC, L], bf16, tag="qt")

        # ------------------------------------------------------------------
        # Phase A: DMA-transposes + Q/K/V projections (per L-superblock)
        # ------------------------------------------------------------------
        with ExitStack() as pa:
            xt_p = pa.enter_context(tc.tile_pool(name="xt", bufs=2))
            w_p = pa.enter_context(tc.tile_pool(name="wstream", bufs=2))
            psA = pa.enter_context(
                tc.tile_pool(name="psA", bufs=1, space="PSUM")
            )

            def transpose_superblock(src_dram, s, xt):
                # xt[:, lc2, hc, c] = src[(s*SBC+lc2)*P + c, hc*P + p]
                for lc2 in range(SBC):
                    row0 = (s * SBC + lc2) * P
                    nc.sync.dma_start_transpose(
                        xt[:, lc2], src_dram[row0 : row0 + P, :]
                    )

            for s in range(NSB):
                xt = xt_p.tile([P, SBC, HC, P], bf16, tag="xt")
                transpose_superblock(xqb_d, s, xt)
                for h in range(NH_LOC):
                    wqt = w_p.tile([P, HC, HD], bf16, tag="w")
                    nc.sync.dma_start(wqt[:], wq_d[h])
                    qtp = psA.tile([P, SB], f32, tag="acc", bufs=3)
                    for hc in range(HC):
                        mm(qtp, wqt[:, hc, :], xt[:, :, hc, :],
                           start=(hc == 0), stop=(hc == HC - 1))
                    nc.scalar.activation(
                        qt_sb[:, h, s * SB : (s + 1) * SB], qtp,
                        mybir.ActivationFunctionType.Identity,
                        bias=bqc[:, h : h + 1],
                    )

                xt2 = xt_p.tile([P, SBC, HC, P], bf16, tag="xt")
                transpose_superblock(xkvb_d, s, xt2)
                for j in range(NKV_LOC):
                    wkt = w_p.tile([P, HC, HD], bf16, tag="w")
                    nc.sync.dma_start(wkt[:], wk_d[j])
                    ktp = psA.tile([P, SB], f32, tag="acc", bufs=3)
                    for hc in range(HC):
                        mm(ktp, wkt[:, hc, :], xt2[:, :, hc, :],
                           start=(hc == 0), stop=(hc == HC - 1))
                    nc.scalar.activation(
                        kt_sb[:, j, s * SB : (s + 1) * SB], ktp,
                        mybir.ActivationFunctionType.Identity,
                        bias=bkc[:, j : j + 1],
                    )
                wvt = w_p.tile([P, HC, NKV_LOC * HD], bf16, tag="w")
                nc.sync.dma_start(wvt[:], wv_d[:])
                for lc2 in range(SBC):
                    vp = psA.tile([P, NKV_LOC * HD], f32, tag="acc", bufs=3)
                    mm(vp, ones_t[:], bvp[:], start=True, stop=False)
                    for hc in range(HC):
                        mm(vp, xt2[:, lc2, hc, :], wvt[:, hc, :],
                           start=False, stop=(hc == HC - 1))
                    kc = s * SBC + lc2
                    for j in range(NKV_LOC):
                        nc.vector.tensor_copy(
                            vaug[:, j, kc, 0:HD],
                            vp[:, j * HD : (j + 1) * HD],
                        )

        # ------------------------------------------------------------------
        # Phase B: attention per (head, 512-query block)
        # ------------------------------------------------------------------
        ot_pool = top.enter_context(tc.tile_pool(name="otp", bufs=1))
        ot_sb = ot_pool.tile([P, NH_LOC, L], bf16, tag="ot")
        with ExitStack() as pb:
            pt_p = pb.enter_context(tc.tile_pool(name="pt", bufs=3))
            osb_p = pb.enter_context(tc.tile_pool(name="osb", bufs=3))
            rcp_p = pb.enter_context(tc.tile_pool(name="rcp", bufs=3))
            psB = pb.enter_context(
                tc.tile_pool(name="psB", bufs=1, space="PSUM")
            )

            QBLK = 512
            NQB = L // QBLK
            QS = QBLK // P
            for h in range(NH_LOC):
                j = h // G
                for qb in range(NQB):
                    o_ps = [
                        psB.tile([P, AVW], f32, tag=f"o{qs}",
                                 name=f"o_ps{qs}")
                        for qs in range(QS)
                    ]
                    for kc in range(KC):
                        stp = psB.tile([P, QBLK], f32, tag="st", bufs=2)
                        mm(stp, kt_sb[:, j, kc * P : (kc + 1) * P],
                           qt_sb[:, h, qb * QBLK : (qb + 1) * QBLK],
                           start=True, stop=True)
                        pt = pt_p.tile([P, QBLK], bf16, tag="pt")
                        nc.scalar.activation(
                            pt, stp,
                            mybir.ActivationFunctionType.Exp,
                            scale=scale,
                        )
                        for qs in range(QS):
                            mm(o_ps[qs], pt[:, qs * P : (qs + 1) * P],
                               vaug[:, j, kc, :],
                               start=(kc == 0), stop=(kc == KC - 1))
                    tps = psB.tile([P, QBLK], bf16, tag="ott")
                    for qs in range(QS):
                        rcp = rcp_p.tile([P, 1], f32, tag="rcp")
                        nc.vector.reciprocal(
                            rcp, o_ps[qs][:, HD : HD + 1]
                        )
                        osb = osb_p.tile([P, HD], bf16, tag="osb")
                        nc.vector.tensor_scalar_mul(
                            osb, o_ps[qs][:, 0:HD], rcp
                        )
                        nc.tensor.matmul(
                            tps[:, qs * P : (qs + 1) * P],
                            osb, ident,
                            is_transpose=True, start=True, stop=True,
                        )
                    nc.vector.tensor_copy(
                        ot_sb[:, h, qb * QBLK : (qb + 1) * QBLK], tps
                    )

        # ------------------------------------------------------------------
        # Phase C: Y = O @ Wo  (partial output; host sums across cores)
        # ------------------------------------------------------------------
        with ExitStack() as pc:
            wo_p = pc.enter_context(tc.tile_pool(name="wo", bufs=2))
            y_p = pc.enter_context(tc.tile_pool(name="ysb", bufs=3))
            psC = pc.enter_context(
                tc.tile_pool(name="psC", bufs=2, space="PSUM")
            )
            for hb in range(HID // SB):
                wot = wo_p.tile([P, NH_LOC, SB], bf16, tag="wo")
                nc.sync.dma_start(wot[:], wo_d[hb])
                for qc in range(LC):
                    yp = psC.tile([P, SB], f32, tag="y")
                    for h in range(NH_LOC):
                        mm(yp, ot_sb[:, h, qc * P : (qc + 1) * P],
                           wot[:, h, :],
                           start=(h == 0), stop=(h == NH_LOC - 1))
                    ysb = y_p.tile([P, SB], f32, tag="y")
                    nc.any.tensor_copy(ysb, yp)
                    nc.sync.dma_start(
                        y_d[qc * P : (qc + 1) * P,
                            hb * SB : (hb + 1) * SB],
                        ysb,
                    )

    nc.compile()
    return nc


# ---------------------------------------------------------------------------
# Host-side sharding / unsharding
# ---------------------------------------------------------------------------
def shard_inputs(query, kv, Wq, bq, Wkv, bkv, Wo, bo):
    """Slice + rearrange + bf16-cast the full inputs into 8 per-core maps."""
    import ml_dtypes

    bf16 = ml_dtypes.bfloat16
    HC = HID // P
    NDQ = NH_LOC * HD  # 1024 local q dims
    NDK = NKV_LOC * HD  # 256 local kv dims
    V_OFF = NKV_LOC * 4 * HD  # start of the V section in Wkv (1024)
    eye = np.eye(P, dtype=bf16)
    ones = np.ones((P, P), dtype=bf16)
    in_maps = []
    for core in range(N_CORES):
        b, g = core // 4, core % 4
        wq_s = Wq[:, g * NDQ : (g + 1) * NDQ]
        wk_s = Wkv[:, g * NDK : (g + 1) * NDK]
        wv_s = Wkv[:, V_OFF + g * NDK : V_OFF + (g + 1) * NDK]
        wo_s = Wo[g * NDQ : (g + 1) * NDQ, :]
        bq_s = bq[g * NDQ : (g + 1) * NDQ]
        bk_s = bkv[g * NDK : (g + 1) * NDK]
        bv_s = bkv[V_OFF + g * NDK : V_OFF + (g + 1) * NDK]

        wq_r = np.ascontiguousarray(
            wq_s.reshape(HC, P, NH_LOC, HD).transpose(2, 1, 0, 3)
        ).astype(bf16)
        wk_r = np.ascontiguousarray(
            wk_s.reshape(HC, P, NKV_LOC, HD).transpose(2, 1, 0, 3)
        ).astype(bf16)
        wv_r = np.ascontiguousarray(
            wv_s.reshape(HC, P, NDK).transpose(1, 0, 2)
        ).astype(bf16)
        wo_r = np.ascontiguousarray(
            wo_s.reshape(NH_LOC, P, HID // SB, SB).transpose(2, 1, 0, 3)
        ).astype(bf16)
        bqc = np.ascontiguousarray(bq_s.reshape(NH_LOC, P).T.astype(np.float32))
        bkc = np.ascontiguousarray(bk_s.reshape(NKV_LOC, P).T.astype(np.float32))
        bvp = np.zeros((P, NDK), bf16)
        bvp[0, :] = bv_s.astype(bf16)

        in_maps.append(
            {
                "xqb": query[b].astype(bf16),
                "xkvb": kv[b].astype(bf16),
                "wq": wq_r,
                "wk": wk_r,
                "wv": wv_r,
                "wo": wo_r,
                "bqc": bqc,
                "bkc": bkc,
                "bvp": bvp,
                "ones": ones,
                "ident": eye,
            }
        )
    return in_maps


def unshard_output(results, bo):
    """Sum the 4 row-parallel partials per batch and add bo."""
    out = np.empty((2, L, HID), np.float32)
    for b in range(2):
        acc = results[b * 4]["y"].astype(np.float32)
        for g in range(1, 4):
            acc = acc + results[b * 4 + g]["y"]
        out[b] = acc + bo.astype(np.float32)[None, :]
    return out


_NC_CACHE = {}


def run_sharded(query, kv, Wq, bq, Wkv, bkv, Wo, bo, trace=False, tmpdir=None,
                **build_kwargs):
    """Shard, run on 8 cores, unshard.  Returns (output, BassKernelResults)."""
    from concourse.bass_utils import run_bass_kernel_spmd

    key = tuple(sorted(build_kwargs.items()))
    if key not in _NC_CACHE:
        _NC_CACHE[key] = build_attention_kernel(**build_kwargs)
    nc = _NC_CACHE[key]

    in_maps = shard_inputs(
        np.asarray(query, np.float32),
        np.asarray(kv, np.float32),
        np.asarray(Wq, np.float32),
        np.asarray(bq, np.float32),
        np.asarray(Wkv, np.float32),
        np.asarray(bkv, np.float32),
        np.asarray(Wo, np.float32),
        np.asarray(bo, np.float32),
    )
    res = run_bass_kernel_spmd(
        nc, in_maps, list(range(N_CORES)), trace=trace, tmpdir=tmpdir
    )
    return unshard_output(res.results, np.asarray(bo, np.float32)), res


def kernel(query, kv, Wq, bq, Wkv, bkv, Wo, bo):
    out, _ = run_sharded(query, kv, Wq, bq, Wkv, bkv, Wo, bo)
    return out


# revision 15
# speedup vs baseline: 1.5039x; 1.1037x over previous
"""Grouped-KV attention block (dense transformer) on 8 Trainium2 NeuronCores.

Sharding (Megatron-style, per the hint): data-parallel over batch (2) x
tensor-parallel over KV-head groups (4).  core = b*4 + g owns batch b and
KV heads {2g, 2g+1} with their 8 query heads (Wq/Wkv column-sharded,
Wo row-sharded).  Each core produces a partial [L, HID] output; the host
sums the 4 partials per batch and adds bo (the row-parallel reduction).

Per-core kernel, bf16 operands / fp32 PSUM accumulation throughout
(4-byte matmul operands stream at half rate on TRN2, so fp32/f32r
compute leaves ~2x on the table):
  A) X^T via XBAR DMA-transpose of host-precast bf16 X (no PE/DVE work);
     Q^T = Wq^T X^T (resident), K^T = Wk^T X^T, V = X Wv (+bias via a
     ones-row matmul).
  B) per (head, 512-query block): S^T = K Q^T per 128-key chunk (PSUM
     fp32), exp on ScalarE -> bf16 probs, O[q,129] += P^T.T @ [V | 1] --
     column 128 accumulates the softmax denominator.  Normalize rows by
     1/denom (per-partition scalar), PE-transpose to O^T.
  C) Y = O @ Wo (accumulate over the 8 local head chunks), DMA out fp32.
"""

import math

import numpy as np

P = 128
HID = 4096
L = 2048
NH_LOC = 8  # query heads per core
NKV_LOC = 2  # kv heads per core
HD = 128  # head dim
SB = 512  # L superblock / matmul free dim
N_CORES = 8


def build_attention_kernel(L=L, HID=HID, NH_LOC=NH_LOC, NKV_LOC=NKV_LOC,
                           HD=HD):
    """Build the per-core Bacc module (compiled, ready for SPMD run)."""
    import concourse.bacc as bacc
    import concourse.mybir as mybir
    import concourse.tile as tile
    from contextlib import ExitStack

    assert L % SB == 0 and HID % P == 0 and HD == P
    HC = HID // P
    LC = L // P
    NSB = L // SB
    SBC = SB // P
    KC = L // P  # key chunks
    G = NH_LOC // NKV_LOC
    f32 = mybir.dt.float32
    bf16 = mybir.dt.bfloat16
    AVW = HD + 1
    scale = 1.0 / math.sqrt(HD)

    nc = bacc.Bacc("TRN2", target_bir_lowering=False, debug=False,
                   num_devices=N_CORES)

    # host-precast bf16 activations and pre-rearranged bf16 weights:
    #   wq [NH, P, HC, HD]; wk [NKV, P, HC, HD]; wv [P, HC, NKV*HD];
    #   wo [HID//SB, P, NH, SB]
    # host-pretransposed X^T: [HID, L] viewed as [P, HC, L] on load
    xqb_d = nc.dram_tensor("xqb", [HID, L], bf16, kind="ExternalInput")
    xkvb_d = nc.dram_tensor("xkvb", [HID, L], bf16, kind="ExternalInput")
    wq_d = nc.dram_tensor("wq", [NH_LOC, P, HC, HD], bf16, kind="ExternalInput")
    wk_d = nc.dram_tensor("wk", [NKV_LOC, P, HC, HD], bf16,
                          kind="ExternalInput")
    wv_d = nc.dram_tensor("wv", [P, HC, NKV_LOC * HD], bf16,
                          kind="ExternalInput")
    wo_d = nc.dram_tensor("wo", [HID // SB, P, NH_LOC, SB], bf16,
                          kind="ExternalInput")
    bqc_d = nc.dram_tensor("bqc", [P, NH_LOC], f32, kind="ExternalInput")
    bkc_d = nc.dram_tensor("bkc", [P, NKV_LOC], f32, kind="ExternalInput")
    bvp_d = nc.dram_tensor("bvp", [P, NKV_LOC * HD], bf16,
                           kind="ExternalInput")
    ones_d = nc.dram_tensor("ones", [P, P], bf16, kind="ExternalInput")
    ident_d = nc.dram_tensor("ident", [P, P], bf16, kind="ExternalInput")
    y_d = nc.dram_tensor("y", [L, HID], f32, kind="ExternalOutput")

    def mm(out, lhsT, rhs, start, stop):
        nc.tensor.matmul(out, lhsT, rhs, start=start, stop=stop)

    with tile.TileContext(nc) as tc, ExitStack() as top:
        consts = top.enter_context(tc.tile_pool(name="consts", bufs=1))
        persist = top.enter_context(tc.tile_pool(name="persist", bufs=1))

        ident = consts.tile([P, P], bf16)
        nc.sync.dma_start(ident[:], ident_d[:])
        ones_t = consts.tile([P, P], bf16)
        nc.sync.dma_start(ones_t[:], ones_d[:])
        bqc = consts.tile([P, NH_LOC], f32)
        nc.sync.dma_start(bqc[:], bqc_d[:])
        bkc = consts.tile([P, NKV_LOC], f32)
        nc.sync.dma_start(bkc[:], bkc_d[:])
        bvp = consts.tile([P, NKV_LOC * HD], bf16)
        nc.sync.dma_start(bvp[:], bvp_d[:])

        kt_sb = persist.tile([P, NKV_LOC, L], bf16, tag="kt")
        vaug = persist.tile([P, NKV_LOC, KC, AVW], bf16, tag="vaug")
        nc.gpsimd.memset(vaug[:, :, :, HD : HD + 1], 1.0)
        qt_sb = persist.tile([P, NH_LOC, L], bf16, tag="qt")

        # ------------------------------------------------------------------
        # Phase A: DMA-transposes + Q/K/V projections (per L-superblock)
        # ------------------------------------------------------------------
        with ExitStack() as pa:
            xt_p = pa.enter_context(tc.tile_pool(name="xt", bufs=2))
            w_p = pa.enter_context(tc.tile_pool(name="wstream", bufs=2))
            psA = pa.enter_context(
                tc.tile_pool(name="psA", bufs=1, space="PSUM")
            )

            def load_xt_superblock(src_dram, s, xt):
                # xt[p, hc, c] = X^T[hc*P + p, s*SB + c]
                nc.sync.dma_start(
                    xt[:],
                    src_dram.rearrange("(hc p) l -> p hc l", p=P)[
                        :, :, s * SB : (s + 1) * SB
                    ],
                )

            for s in range(NSB):
                xt = xt_p.tile([P, HC, SB], bf16, tag="xt")
                load_xt_superblock(xqb_d, s, xt)
                for h in range(NH_LOC):
                    wqt = w_p.tile([P, HC, HD], bf16, tag="w")
                    nc.sync.dma_start(wqt[:], wq_d[h])
                    qtp = psA.tile([P, SB], f32, tag="acc", bufs=3)
                    for hc in range(HC):
                        mm(qtp, wqt[:, hc, :], xt[:, hc, :],
                           start=(hc == 0), stop=(hc == HC - 1))
                    nc.scalar.activation(
                        qt_sb[:, h, s * SB : (s + 1) * SB], qtp,
                        mybir.ActivationFunctionType.Identity,
                        bias=bqc[:, h : h + 1],
                    )

                xt2 = xt_p.tile([P, HC, SB], bf16, tag="xt")
                load_xt_superblock(xkvb_d, s, xt2)
                for j in range(NKV_LOC):
                    wkt = w_p.tile([P, HC, HD], bf16, tag="w")
                    nc.sync.dma_start(wkt[:], wk_d[j])
                    ktp = psA.tile([P, SB], f32, tag="acc", bufs=3)
                    for hc in range(HC):
                        mm(ktp, wkt[:, hc, :], xt2[:, hc, :],
                           start=(hc == 0), stop=(hc == HC - 1))
                    nc.scalar.activation(
                        kt_sb[:, j, s * SB : (s + 1) * SB], ktp,
                        mybir.ActivationFunctionType.Identity,
                        bias=bkc[:, j : j + 1],
                    )
                wvt = w_p.tile([P, HC, NKV_LOC * HD], bf16, tag="w")
                nc.sync.dma_start(wvt[:], wv_d[:])
                for lc2 in range(SBC):
                    vp = psA.tile([P, NKV_LOC * HD], f32, tag="acc", bufs=3)
                    mm(vp, ones_t[:], bvp[:], start=True, stop=False)
                    for hc in range(HC):
                        mm(vp, xt2[:, hc, lc2 * P : (lc2 + 1) * P],
                           wvt[:, hc, :],
                           start=False, stop=(hc == HC - 1))
                    kc = s * SBC + lc2
                    for j in range(NKV_LOC):
                        nc.vector.tensor_copy(
                            vaug[:, j, kc, 0:HD],
                            vp[:, j * HD : (j + 1) * HD],
                        )

        # ------------------------------------------------------------------
        # Phase B: attention per (head, 512-query block)
        # ------------------------------------------------------------------
        ot_pool = top.enter_context(tc.tile_pool(name="otp", bufs=1))
        ot_sb = ot_pool.tile([P, NH_LOC, L], bf16, tag="ot")
        with ExitStack() as pb:
            pt_p = pb.enter_context(tc.tile_pool(name="pt", bufs=3))
            osb_p = pb.enter_context(tc.tile_pool(name="osb", bufs=3))
            rcp_p = pb.enter_context(tc.tile_pool(name="rcp", bufs=3))
            psB = pb.enter_context(
                tc.tile_pool(name="psB", bufs=1, space="PSUM")
            )

            QBLK = 512
            NQB = L // QBLK
            QS = QBLK // P
            for h in range(NH_LOC):
                j = h // G
                for qb in range(NQB):
                    o_ps = [
                        psB.tile([P, AVW], f32, tag=f"o{qs}",
                                 name=f"o_ps{qs}")
                        for qs in range(QS)
                    ]
                    for kc in range(KC):
                        stp = psB.tile([P, QBLK], f32, tag="st", bufs=2)
                        mm(stp, kt_sb[:, j, kc * P : (kc + 1) * P],
                           qt_sb[:, h, qb * QBLK : (qb + 1) * QBLK],
                           start=True, stop=True)
                        pt = pt_p.tile([P, QBLK], bf16, tag="pt")
                        nc.scalar.activation(
                            pt, stp,
                            mybir.ActivationFunctionType.Exp,
                            scale=scale,
                        )
                        for qs in range(QS):
                            mm(o_ps[qs], pt[:, qs * P : (qs + 1) * P],
                               vaug[:, j, kc, :],
                               start=(kc == 0), stop=(kc == KC - 1))
                    tps = psB.tile([P, QBLK], bf16, tag="ott")
                    for qs in range(QS):
                        rcp = rcp_p.tile([P, 1], f32, tag="rcp")
                        nc.vector.reciprocal(
                            rcp, o_ps[qs][:, HD : HD + 1]
                        )
                        osb = osb_p.tile([P, HD], bf16, tag="osb")
                        nc.vector.tensor_scalar_mul(
                            osb, o_ps[qs][:, 0:HD], rcp
                        )
                        nc.tensor.matmul(
                            tps[:, qs * P : (qs + 1) * P],
                            osb, ident,
                            is_transpose=True, start=True, stop=True,
                        )
                    nc.vector.tensor_copy(
                        ot_sb[:, h, qb * QBLK : (qb + 1) * QBLK], tps
                    )

        # ------------------------------------------------------------------
        # Phase C: Y = O @ Wo  (partial output; host sums across cores)
        # ------------------------------------------------------------------
        with ExitStack() as pc:
            wo_p = pc.enter_context(tc.tile_pool(name="wo", bufs=2))
            y_p = pc.enter_context(tc.tile_pool(name="ysb", bufs=3))
            psC = pc.enter_context(
                tc.tile_pool(name="psC", bufs=2, space="PSUM")
            )
            for hb in range(HID // SB):
                wot = wo_p.tile([P, NH_LOC, SB], bf16, tag="wo")
                nc.sync.dma_start(wot[:], wo_d[hb])
                for qc in range(LC):
                    yp = psC.tile([P, SB], f32, tag="y")
                    for h in range(NH_LOC):
                        mm(yp, ot_sb[:, h, qc * P : (qc + 1) * P],
                           wot[:, h, :],
                           start=(h == 0), stop=(h == NH_LOC - 1))
                    ysb = y_p.tile([P, SB], f32, tag="y")
                    nc.vector.tensor_copy(ysb, yp)
                    nc.sync.dma_start(
                        y_d[qc * P : (qc + 1) * P,
                            hb * SB : (hb + 1) * SB],
                        ysb,
                    )

    nc.compile()
    return nc


# ---------------------------------------------------------------------------
# Host-side sharding / unsharding
# ---------------------------------------------------------------------------
def shard_inputs(query, kv, Wq, bq, Wkv, bkv, Wo, bo):
    """Slice + rearrange + bf16-cast the full inputs into 8 per-core maps."""
    import ml_dtypes

    bf16 = ml_dtypes.bfloat16
    HC = HID // P
    NDQ = NH_LOC * HD  # 1024 local q dims
    NDK = NKV_LOC * HD  # 256 local kv dims
    V_OFF = NKV_LOC * 4 * HD  # start of the V section in Wkv (1024)
    eye = np.eye(P, dtype=bf16)
    ones = np.ones((P, P), dtype=bf16)
    in_maps = []
    for core in range(N_CORES):
        b, g = core // 4, core % 4
        wq_s = Wq[:, g * NDQ : (g + 1) * NDQ]
        wk_s = Wkv[:, g * NDK : (g + 1) * NDK]
        wv_s = Wkv[:, V_OFF + g * NDK : V_OFF + (g + 1) * NDK]
        wo_s = Wo[g * NDQ : (g + 1) * NDQ, :]
        bq_s = bq[g * NDQ : (g + 1) * NDQ]
        bk_s = bkv[g * NDK : (g + 1) * NDK]
        bv_s = bkv[V_OFF + g * NDK : V_OFF + (g + 1) * NDK]

        wq_r = np.ascontiguousarray(
            wq_s.reshape(HC, P, NH_LOC, HD).transpose(2, 1, 0, 3)
        ).astype(bf16)
        wk_r = np.ascontiguousarray(
            wk_s.reshape(HC, P, NKV_LOC, HD).transpose(2, 1, 0, 3)
        ).astype(bf16)
        wv_r = np.ascontiguousarray(
            wv_s.reshape(HC, P, NDK).transpose(1, 0, 2)
        ).astype(bf16)
        wo_r = np.ascontiguousarray(
            wo_s.reshape(NH_LOC, P, HID // SB, SB).transpose(2, 1, 0, 3)
        ).astype(bf16)
        bqc = np.ascontiguousarray(bq_s.reshape(NH_LOC, P).T.astype(np.float32))
        bkc = np.ascontiguousarray(bk_s.reshape(NKV_LOC, P).T.astype(np.float32))
        bvp = np.zeros((P, NDK), bf16)
        bvp[0, :] = bv_s.astype(bf16)

        in_maps.append(
            {
                "xqb": np.ascontiguousarray(query[b].T).astype(bf16),
                "xkvb": np.ascontiguousarray(kv[b].T).astype(bf16),
                "wq": wq_r,
                "wk": wk_r,
                "wv": wv_r,
                "wo": wo_r,
                "bqc": bqc,
                "bkc": bkc,
                "bvp": bvp,
                "ones": ones,
                "ident": eye,
            }
        )
    return in_maps


def unshard_output(results, bo):
    """Sum the 4 row-parallel partials per batch and add bo."""
    out = np.empty((2, L, HID), np.float32)
    for b in range(2):
        acc = results[b * 4]["y"].astype(np.float32)
        for g in range(1, 4):
            acc = acc + results[b * 4 + g]["y"]
        out[b] = acc + bo.astype(np.float32)[None, :]
    return out


_NC_CACHE = {}


def run_sharded(query, kv, Wq, bq, Wkv, bkv, Wo, bo, trace=False, tmpdir=None,
                **build_kwargs):
    """Shard, run on 8 cores, unshard.  Returns (output, BassKernelResults)."""
    from concourse.bass_utils import run_bass_kernel_spmd

    key = tuple(sorted(build_kwargs.items()))
    if key not in _NC_CACHE:
        _NC_CACHE[key] = build_attention_kernel(**build_kwargs)
    nc = _NC_CACHE[key]

    in_maps = shard_inputs(
        np.asarray(query, np.float32),
        np.asarray(kv, np.float32),
        np.asarray(Wq, np.float32),
        np.asarray(bq, np.float32),
        np.asarray(Wkv, np.float32),
        np.asarray(bkv, np.float32),
        np.asarray(Wo, np.float32),
        np.asarray(bo, np.float32),
    )
    res = run_bass_kernel_spmd(
        nc, in_maps, list(range(N_CORES)), trace=trace, tmpdir=tmpdir
    )
    return unshard_output(res.results, np.asarray(bo, np.float32)), res


def kernel(query, kv, Wq, bq, Wkv, bkv, Wo, bo):
    out, _ = run_sharded(query, kv, Wq, bq, Wkv, bkv, Wo, bo)
    return out


# revision 17
# speedup vs baseline: 1.5543x; 1.0335x over previous
"""Grouped-KV attention block (dense transformer) on 8 Trainium2 NeuronCores.

Sharding (Megatron-style, per the hint): data-parallel over batch (2) x
tensor-parallel over KV-head groups (4).  core = b*4 + g owns batch b and
KV heads {2g, 2g+1} with their 8 query heads (Wq/Wkv column-sharded,
Wo row-sharded).  Each core produces a partial [L, HID] output; the host
sums the 4 partials per batch and adds bo (the row-parallel reduction).

Per-core kernel, bf16 operands / fp32 PSUM accumulation throughout
(4-byte matmul operands stream at half rate on TRN2, so fp32/f32r
compute leaves ~2x on the table):
  A) X^T via XBAR DMA-transpose of host-precast bf16 X (no PE/DVE work);
     Q^T = Wq^T X^T (resident), K^T = Wk^T X^T, V = X Wv (+bias via a
     ones-row matmul).
  B) per (head, 512-query block): S^T = K Q^T per 128-key chunk (PSUM
     fp32), exp on ScalarE -> bf16 probs, O[q,129] += P^T.T @ [V | 1] --
     column 128 accumulates the softmax denominator.  Normalize rows by
     1/denom (per-partition scalar), PE-transpose to O^T.
  C) Y = O @ Wo (accumulate over the 8 local head chunks), DMA out fp32.
"""

import math

import numpy as np

P = 128
HID = 4096
L = 2048
NH_LOC = 8  # query heads per core
NKV_LOC = 2  # kv heads per core
HD = 128  # head dim
SB = 512  # L superblock / matmul free dim
N_CORES = 8


def build_attention_kernel(L=L, HID=HID, NH_LOC=NH_LOC, NKV_LOC=NKV_LOC,
                           HD=HD):
    """Build the per-core Bacc module (compiled, ready for SPMD run)."""
    import concourse.bacc as bacc
    import concourse.mybir as mybir
    import concourse.tile as tile
    from contextlib import ExitStack

    assert L % SB == 0 and HID % P == 0 and HD == P
    HC = HID // P
    LC = L // P
    NSB = L // SB
    SBC = SB // P
    KC = L // P  # key chunks
    G = NH_LOC // NKV_LOC
    f32 = mybir.dt.float32
    bf16 = mybir.dt.bfloat16
    AVW = HD + 1
    scale = 1.0 / math.sqrt(HD)

    nc = bacc.Bacc("TRN2", target_bir_lowering=False, debug=False,
                   num_devices=N_CORES)

    # host-precast bf16 activations and pre-rearranged bf16 weights:
    #   wq [NH, P, HC, HD]; wk [NKV, P, HC, HD]; wv [P, HC, NKV*HD];
    #   wo [HID//SB, P, NH, SB]
    # host-pretransposed X^T: [HID, L] viewed as [P, HC, L] on load
    xqb_d = nc.dram_tensor("xqb", [HID, L], bf16, kind="ExternalInput")
    xkvb_d = nc.dram_tensor("xkvb", [HID, L], bf16, kind="ExternalInput")
    wq_d = nc.dram_tensor("wq", [NH_LOC, P, HC, HD], bf16, kind="ExternalInput")
    wk_d = nc.dram_tensor("wk", [NKV_LOC, P, HC, HD], bf16,
                          kind="ExternalInput")
    wv_d = nc.dram_tensor("wv", [P, HC, NKV_LOC * HD], bf16,
                          kind="ExternalInput")
    wo_d = nc.dram_tensor("wo", [HID // SB, P, NH_LOC, SB], bf16,
                          kind="ExternalInput")
    bqc_d = nc.dram_tensor("bqc", [P, NH_LOC], f32, kind="ExternalInput")
    bkc_d = nc.dram_tensor("bkc", [P, NKV_LOC], f32, kind="ExternalInput")
    bvp_d = nc.dram_tensor("bvp", [P, NKV_LOC * HD], bf16,
                           kind="ExternalInput")
    ones_d = nc.dram_tensor("ones", [P, P], bf16, kind="ExternalInput")
    ident_d = nc.dram_tensor("ident", [P, P], bf16, kind="ExternalInput")
    y_d = nc.dram_tensor("y", [L, HID], f32, kind="ExternalOutput")

    def mm(out, lhsT, rhs, start, stop):
        nc.tensor.matmul(out, lhsT, rhs, start=start, stop=stop)

    with tile.TileContext(nc) as tc, ExitStack() as top:
        consts = top.enter_context(tc.tile_pool(name="consts", bufs=1))
        persist = top.enter_context(tc.tile_pool(name="persist", bufs=1))

        ident = consts.tile([P, P], bf16)
        nc.sync.dma_start(ident[:], ident_d[:])
        ones_t = consts.tile([P, P], bf16)
        nc.sync.dma_start(ones_t[:], ones_d[:])
        bqc = consts.tile([P, NH_LOC], f32)
        nc.sync.dma_start(bqc[:], bqc_d[:])
        bkc = consts.tile([P, NKV_LOC], f32)
        nc.sync.dma_start(bkc[:], bkc_d[:])
        bvp = consts.tile([P, NKV_LOC * HD], bf16)
        nc.sync.dma_start(bvp[:], bvp_d[:])

        kt_sb = persist.tile([P, NKV_LOC, L], bf16, tag="kt")
        vaug = persist.tile([P, NKV_LOC, KC, AVW], bf16, tag="vaug")
        nc.gpsimd.memset(vaug[:, :, :, HD : HD + 1], 1.0)
        qt_sb = persist.tile([P, NH_LOC, L], bf16, tag="qt")

        # ------------------------------------------------------------------
        # Phase A: DMA-transposes + Q/K/V projections (per L-superblock)
        # ------------------------------------------------------------------
        with ExitStack() as pa:
            xt_p = pa.enter_context(tc.tile_pool(name="xt", bufs=2))
            w_p = pa.enter_context(tc.tile_pool(name="wstream", bufs=2))
            psA = pa.enter_context(
                tc.tile_pool(name="psA", bufs=1, space="PSUM")
            )

            def load_xt_superblock(src_dram, s, xt):
                # xt[p, hc, c] = X^T[hc*P + p, s*SB + c]
                nc.sync.dma_start(
                    xt[:],
                    src_dram.rearrange("(hc p) l -> p hc l", p=P)[
                        :, :, s * SB : (s + 1) * SB
                    ],
                )

            for s in range(NSB):
                xt = xt_p.tile([P, HC, SB], bf16, tag="xt")
                load_xt_superblock(xqb_d, s, xt)
                for h in range(NH_LOC):
                    wqt = w_p.tile([P, HC, HD], bf16, tag="w")
                    nc.sync.dma_start(wqt[:], wq_d[h])
                    qtp = psA.tile([P, SB], f32, tag="acc", bufs=3)
                    for hc in range(HC):
                        mm(qtp, wqt[:, hc, :], xt[:, hc, :],
                           start=(hc == 0), stop=(hc == HC - 1))
                    nc.scalar.activation(
                        qt_sb[:, h, s * SB : (s + 1) * SB], qtp,
                        mybir.ActivationFunctionType.Identity,
                        bias=bqc[:, h : h + 1],
                    )

                xt2 = xt_p.tile([P, HC, SB], bf16, tag="xt")
                load_xt_superblock(xkvb_d, s, xt2)
                for j in range(NKV_LOC):
                    wkt = w_p.tile([P, HC, HD], bf16, tag="w")
                    nc.sync.dma_start(wkt[:], wk_d[j])
                    ktp = psA.tile([P, SB], f32, tag="acc", bufs=3)
                    for hc in range(HC):
                        mm(ktp, wkt[:, hc, :], xt2[:, hc, :],
                           start=(hc == 0), stop=(hc == HC - 1))
                    nc.scalar.activation(
                        kt_sb[:, j, s * SB : (s + 1) * SB], ktp,
                        mybir.ActivationFunctionType.Identity,
                        bias=bkc[:, j : j + 1],
                    )
                wvt = w_p.tile([P, HC, NKV_LOC * HD], bf16, tag="w")
                nc.sync.dma_start(wvt[:], wv_d[:])
                for lc2 in range(SBC):
                    vp = psA.tile([P, NKV_LOC * HD], f32, tag="acc", bufs=3)
                    mm(vp, ones_t[:], bvp[:], start=True, stop=False)
                    for hc in range(HC):
                        mm(vp, xt2[:, hc, lc2 * P : (lc2 + 1) * P],
                           wvt[:, hc, :],
                           start=False, stop=(hc == HC - 1))
                    kc = s * SBC + lc2
                    for j in range(NKV_LOC):
                        nc.vector.tensor_copy(
                            vaug[:, j, kc, 0:HD],
                            vp[:, j * HD : (j + 1) * HD],
                        )

        # ------------------------------------------------------------------
        # Phases B+C fused per 512-query block: attention for all heads,
        # then immediately Y = O @ Wo for this block (overlaps the
        # ACT-bound exp stretch of the next block's attention).
        # ------------------------------------------------------------------
        wo_pool = top.enter_context(tc.tile_pool(name="wop", bufs=1))
        wo_sb = wo_pool.tile([P, NH_LOC, HID], bf16, tag="wo")
        for hb in range(HID // SB):
            nc.sync.dma_start(
                wo_sb[:, :, hb * SB : (hb + 1) * SB], wo_d[hb]
            )
        with ExitStack() as pb:
            ot_p = pb.enter_context(tc.tile_pool(name="otq", bufs=2))
            pt_p = pb.enter_context(tc.tile_pool(name="pt", bufs=3))
            osb_p = pb.enter_context(tc.tile_pool(name="osb", bufs=3))
            rcp_p = pb.enter_context(tc.tile_pool(name="rcp", bufs=3))
            y_p = pb.enter_context(tc.tile_pool(name="ysb", bufs=3))
            psB = pb.enter_context(
                tc.tile_pool(name="psB", bufs=1, space="PSUM")
            )

            QBLK = 512
            NQB = L // QBLK
            QS = QBLK // P
            for qb in range(NQB):
                ot_qb = ot_p.tile([P, NH_LOC, QBLK], bf16, tag="otq")
                for h in range(NH_LOC):
                    j = h // G
                    o_ps = [
                        psB.tile([P, AVW], f32, tag=f"o{qs}",
                                 name=f"o_ps{qs}")
                        for qs in range(QS)
                    ]
                    for kc in range(KC):
                        stp = psB.tile([P, QBLK], f32, tag="st", bufs=2)
                        mm(stp, kt_sb[:, j, kc * P : (kc + 1) * P],
                           qt_sb[:, h, qb * QBLK : (qb + 1) * QBLK],
                           start=True, stop=True)
                        pt = pt_p.tile([P, QBLK], bf16, tag="pt")
                        nc.scalar.activation(
                            pt, stp,
                            mybir.ActivationFunctionType.Exp,
                            scale=scale,
                        )
                        for qs in range(QS):
                            mm(o_ps[qs], pt[:, qs * P : (qs + 1) * P],
                               vaug[:, j, kc, :],
                               start=(kc == 0), stop=(kc == KC - 1))
                    tps = psB.tile([P, QBLK], bf16, tag="y", bufs=2, name="tps")
                    for qs in range(QS):
                        rcp = rcp_p.tile([P, 1], f32, tag="rcp")
                        nc.vector.reciprocal(
                            rcp, o_ps[qs][:, HD : HD + 1]
                        )
                        osb = osb_p.tile([P, HD], bf16, tag="osb")
                        nc.vector.tensor_scalar_mul(
                            osb, o_ps[qs][:, 0:HD], rcp
                        )
                        nc.tensor.matmul(
                            tps[:, qs * P : (qs + 1) * P],
                            osb, ident,
                            is_transpose=True, start=True, stop=True,
                        )
                    nc.vector.tensor_copy(ot_qb[:, h, :], tps)

                # Y for this query block
                for hb in range(HID // SB):
                    for qc2 in range(QS):
                        yp = psB.tile([P, SB], f32, tag="y", bufs=2)
                        for h in range(NH_LOC):
                            mm(yp, ot_qb[:, h, qc2 * P : (qc2 + 1) * P],
                               wo_sb[:, h, hb * SB : (hb + 1) * SB],
                               start=(h == 0), stop=(h == NH_LOC - 1))
                        ysb = y_p.tile([P, SB], f32, tag="y")
                        nc.vector.tensor_copy(ysb, yp)
                        nc.sync.dma_start(
                            y_d[(qb * QS + qc2) * P : (qb * QS + qc2 + 1) * P,
                                hb * SB : (hb + 1) * SB],
                            ysb,
                        )

    nc.compile()
    return nc


# ---------------------------------------------------------------------------
# Host-side sharding / unsharding
# ---------------------------------------------------------------------------
def shard_inputs(query, kv, Wq, bq, Wkv, bkv, Wo, bo):
    """Slice + rearrange + bf16-cast the full inputs into 8 per-core maps."""
    import ml_dtypes

    bf16 = ml_dtypes.bfloat16
    HC = HID // P
    NDQ = NH_LOC * HD  # 1024 local q dims
    NDK = NKV_LOC * HD  # 256 local kv dims
    V_OFF = NKV_LOC * 4 * HD  # start of the V section in Wkv (1024)
    eye = np.eye(P, dtype=bf16)
    ones = np.ones((P, P), dtype=bf16)
    in_maps = []
    for core in range(N_CORES):
        b, g = core // 4, core % 4
        wq_s = Wq[:, g * NDQ : (g + 1) * NDQ]
        wk_s = Wkv[:, g * NDK : (g + 1) * NDK]
        wv_s = Wkv[:, V_OFF + g * NDK : V_OFF + (g + 1) * NDK]
        wo_s = Wo[g * NDQ : (g + 1) * NDQ, :]
        bq_s = bq[g * NDQ : (g + 1) * NDQ]
        bk_s = bkv[g * NDK : (g + 1) * NDK]
        bv_s = bkv[V_OFF + g * NDK : V_OFF + (g + 1) * NDK]

        wq_r = np.ascontiguousarray(
            wq_s.reshape(HC, P, NH_LOC, HD).transpose(2, 1, 0, 3)
        ).astype(bf16)
        wk_r = np.ascontiguousarray(
            wk_s.reshape(HC, P, NKV_LOC, HD).transpose(2, 1, 0, 3)
        ).astype(bf16)
        wv_r = np.ascontiguousarray(
            wv_s.reshape(HC, P, NDK).transpose(1, 0, 2)
        ).astype(bf16)
        wo_r = np.ascontiguousarray(
            wo_s.reshape(NH_LOC, P, HID // SB, SB).transpose(2, 1, 0, 3)
        ).astype(bf16)
        bqc = np.ascontiguousarray(bq_s.reshape(NH_LOC, P).T.astype(np.float32))
        bkc = np.ascontiguousarray(bk_s.reshape(NKV_LOC, P).T.astype(np.float32))
        bvp = np.zeros((P, NDK), bf16)
        bvp[0, :] = bv_s.astype(bf16)

        in_maps.append(
            {
                "xqb": np.ascontiguousarray(query[b].T).astype(bf16),
                "xkvb": np.ascontiguousarray(kv[b].T).astype(bf16),
                "wq": wq_r,
                "wk": wk_r,
                "wv": wv_r,
                "wo": wo_r,
                "bqc": bqc,
                "bkc": bkc,
                "bvp": bvp,
                "ones": ones,
                "ident": eye,
            }
        )
    return in_maps


def unshard_output(results, bo):
    """Sum the 4 row-parallel partials per batch and add bo."""
    out = np.empty((2, L, HID), np.float32)
    for b in range(2):
        acc = results[b * 4]["y"].astype(np.float32)
        for g in range(1, 4):
            acc = acc + results[b * 4 + g]["y"]
        out[b] = acc + bo.astype(np.float32)[None, :]
    return out


_NC_CACHE = {}


def run_sharded(query, kv, Wq, bq, Wkv, bkv, Wo, bo, trace=False, tmpdir=None,
                **build_kwargs):
    """Shard, run on 8 cores, unshard.  Returns (output, BassKernelResults)."""
    from concourse.bass_utils import run_bass_kernel_spmd

    key = tuple(sorted(build_kwargs.items()))
    if key not in _NC_CACHE:
        _NC_CACHE[key] = build_attention_kernel(**build_kwargs)
    nc = _NC_CACHE[key]

    in_maps = shard_inputs(
        np.asarray(query, np.float32),
        np.asarray(kv, np.float32),
        np.asarray(Wq, np.float32),
        np.asarray(bq, np.float32),
        np.asarray(Wkv, np.float32),
        np.asarray(bkv, np.float32),
        np.asarray(Wo, np.float32),
        np.asarray(bo, np.float32),
    )
    res = run_bass_kernel_spmd(
        nc, in_maps, list(range(N_CORES)), trace=trace, tmpdir=tmpdir
    )
    return unshard_output(res.results, np.asarray(bo, np.float32)), res


def kernel(query, kv, Wq, bq, Wkv, bkv, Wo, bo):
    out, _ = run_sharded(query, kv, Wq, bq, Wkv, bkv, Wo, bo)
    return out
